# revision 1
# baseline (speedup 1.0000x reference)
"""Trainium2 Bass kernel for nn_Disc_edge_15573551415682 (GNN message passing).

Sharding: data-parallel over batch B=8 -> 8 NeuronCores (1 graph/core).

Device math (per graph, all edge tensors in "pair-tile" layout):
  pair q in [0,128) covers node rows (q, q+128).
  pair-tile = [128 partitions, 256 cols]:
    partitions 0:64   = features of row q      (feature-major)
    partitions 64:128 = features of row q+128
    cols = j (neighbor index)

  Per layer l, per 512-col block g (pairs 2g, 2g+1), PSUM [128,512]:
    MM1: lhsT = BD_l   [128,128] block-diag(We_e ; We_e), rhs = e-tiles
    MM2: lhsT = Wxj2_l [64,128]  (Wxj | Wxj),            rhs = xT tiled x2
    MM3: lhsT = BIG2   [2,128],                          rhs = (A-1) rows
         -> adds (A[i,j]-1)*32768 => relu masks the edge (layers 0,2 only;
            layer 1 garbage in masked cols never crosses columns).
  Eviction (per pair, even->ACT odd->DVE):
    relu(psum + bias_col) -> bf16 e-tile, fused accum_out = row-sums
    (bias_col = Axi[:,i] + be : the sender-node term, constant along j).

Layer 0 input: edge_attr is pre-arranged on the host into the feature-major
pair-tile layout; the device does one contiguous gpsimd cast-DMA (fp32->bf16)
per chunk. x1 (node update) computed on device; mean head MLP on host.
"""

import sys
from contextlib import ExitStack

import numpy as np

sys.path.insert(0, "/opt/trn_rl_repo")

import ml_dtypes  # noqa: E402

import concourse.bacc as bacc  # noqa: E402
import concourse.bass as bass  # noqa: E402
import concourse.tile as tile  # noqa: E402
from concourse import mybir  # noqa: E402
from concourse.bass_utils import run_bass_kernel_spmd  # noqa: E402

BF16 = ml_dtypes.bfloat16
F32 = np.float32

B, N, FN, FE = 8, 256, 64, 64
NPAIR = 128          # pairs (q, q+128)
NBLK = 64            # 512-col blocks (2 pairs each)
QC = 16              # pairs per load chunk (1 MB fp32 per chunk read)
NCHUNK = NPAIR // QC
BIGV = 32768.0

_DT = mybir.dt
_nc_cache = None


def _relu(a):
    return np.maximum(a, 0.0)


def _build_program():
    nc = bacc.Bacc(
        "TRN2", target_bir_lowering=False, debug=False, num_devices=8
    )

    def din(name, shape, dt):
        return nc.dram_tensor(name, shape, dt, kind="ExternalInput").ap()

    def dout(name, shape, dt):
        return nc.dram_tensor(name, shape, dt, kind="ExternalOutput").ap()

    e0d = din("e0", [128, 128 * 256], _DT.float32)
    am1d = din("am1", [2, NPAIR * 256], _DT.bfloat16)
    x0t2d = din("x0t2", [64, 512], _DT.bfloat16)
    bias0d = din("bias0", [128, 128], _DT.float32)
    dinvPd = din("dinvP", [128, 128], _DT.float32)
    bd0d = din("bd0", [128, 128], _DT.bfloat16)
    bd1d = din("bd1", [128, 128], _DT.bfloat16)
    bd2d = din("bd2", [128, 128], _DT.bfloat16)
    w23_0d = din("w23_0", [66, 128], _DT.bfloat16)
    w23r1d = din("w23rep_1", [68, 8192], _DT.bfloat16)
    w23r2d = din("w23rep_2", [68, 8192], _DT.bfloat16)
    ind2d = din("ind2", [2, QC * 256], _DT.bfloat16)
    wxibe1d = din("wxibe1", [65, 64], _DT.bfloat16)
    wxibe2d = din("wxibe2", [65, 64], _DT.bfloat16)
    wn0xd = din("wn0x", [64, 64], _DT.bfloat16)
    wn0ad = din("wn0a", [64, 64], _DT.bfloat16)
    wn0a2d = din("wn0a2", [128, 64], _DT.bfloat16)
    bn0cd = din("bn0c", [64, 1], _DT.float32)

    voutd = dout("vcols", [128, 32], _DT.float32)


    with tile.TileContext(nc) as tc, ExitStack() as ctx:
        cst = ctx.enter_context(tc.tile_pool(name="cst", bufs=1))
        fmp = ctx.enter_context(tc.tile_pool(name="fm", bufs=3))
        pspB = ctx.enter_context(tc.tile_pool(name="psB", bufs=4, space="PSUM"))
        e2p = ctx.enter_context(tc.tile_pool(name="e2s", bufs=4))
        e3p = ctx.enter_context(tc.tile_pool(name="e3s", bufs=4))
        e1pool = ctx.enter_context(tc.tile_pool(name="e1", bufs=1))
        smallp = ctx.enter_context(tc.tile_pool(name="small", bufs=1))

        # ---- constants / weights into SBUF ----
        # first edge chunk starts immediately (SWDGE path, parallel to the
        # HWDGE const loads below) so the PE has work ASAP
        fm0 = fmp.tile([128, QC * 256], _DT.bfloat16, tag="fm", name="fm0")
        half = QC * 256 // 2
        nc.gpsimd.dma_start(fm0[:, 0:half], e0d[:, 0:half])
        nc.gpsimd.dma_start(fm0[:, half:], e0d[:, half : QC * 256])

        def cload(ap_dram, shape, dt, tag):
            t = cst.tile(shape, dt, tag=tag, name=tag)
            nc.sync.dma_start(t[:], ap_dram)
            return t

        x0t2 = cload(x0t2d, [64, 512], _DT.bfloat16, "x0t2")
        bias0 = cload(bias0d, [128, 128], _DT.float32, "bias0")
        dinvP = cload(dinvPd, [128, 128], _DT.float32, "dinvP")
        bd = [
            cload(d, [128, 128], _DT.bfloat16, f"bd{i}")
            for i, d in enumerate([bd0d, bd1d, bd2d])
        ]
        w23_0 = cload(w23_0d, [66, 128], _DT.bfloat16, "w23_0")
        w23r1 = cload(w23r1d, [68, 8192], _DT.bfloat16, "w23r1")
        w23r2 = cload(w23r2d, [68, 8192], _DT.bfloat16, "w23r2")
        wxibe1 = cload(wxibe1d, [65, 64], _DT.bfloat16, "wxibe1")
        wxibe2 = cload(wxibe2d, [65, 64], _DT.bfloat16, "wxibe2")
        wn0x = cload(wn0xd, [64, 64], _DT.bfloat16, "wn0x")
        wn0a = cload(wn0ad, [64, 64], _DT.bfloat16, "wn0a")
        wn0a2 = cload(wn0a2d, [128, 64], _DT.bfloat16, "wn0a2")
        bn0c = cload(bn0cd, [64, 1], _DT.float32, "bn0c")

        zeros = cst.tile([128, 256], _DT.bfloat16, tag="zeros")
        nc.vector.memset(zeros[:], 0.0)

        e1 = e1pool.tile([128, NPAIR * 256], _DT.bfloat16, tag="e1")
        aggP = smallp.tile([128, 128], _DT.float32, tag="aggP")
        vcols = smallp.tile([128, 32], _DT.float32, tag="vcols")
        x1t2 = smallp.tile([64, 512], _DT.bfloat16, tag="x1t2")
        m2r = [
            smallp.tile([68, QC * 256], _DT.bfloat16, tag=f"m2r{s}",
                        name=f"m2r{s}")
            for s in (0, 1)
        ]
        nc.sync.dma_start(m2r[0][66:68, :], ind2d)
        nc.sync.dma_start(m2r[0][64:66, :], am1d[:, 0 : QC * 256])
        nc.sync.dma_start(m2r[1][66:68, :], ind2d)
        # remaining (pass-B / transition) constants load behind pass-A setup
        dinvP = cload(dinvPd, [128, 128], _DT.float32, "dinvP")
        bd[1] = cload(bd1d, [128, 128], _DT.bfloat16, "bd1")
        bd[2] = cload(bd2d, [128, 128], _DT.bfloat16, "bd2")
        wxibe1 = cload(wxibe1d, [65, 64], _DT.bfloat16, "wxibe1")
        wxibe2 = cload(wxibe2d, [65, 64], _DT.bfloat16, "wxibe2")
        wn0x = cload(wn0xd, [64, 64], _DT.bfloat16, "wn0x")
        wn0a = cload(wn0ad, [64, 64], _DT.bfloat16, "wn0a")
        wn0a2 = cload(wn0a2d, [128, 64], _DT.bfloat16, "wn0a2")
        bn0c = cload(bn0cd, [64, 1], _DT.float32, "bn0c")
        w23r1 = cload(w23r1d, [68, 8192], _DT.bfloat16, "w23r1")
        w23r2 = cload(w23r2d, [68, 8192], _DT.bfloat16, "w23r2")
        x1o = smallp.tile([65, 256], _DT.bfloat16, tag="x1o")
        nc.vector.memset(x1o[64:65, :], 1.0)

        AF = mybir.ActivationFunctionType
        ALU = mybir.AluOpType

        def seed_xpart(slot, xt2):
            nc.vector.tensor_copy(slot[0:64, 0:512], xt2[:])
            nc.vector.tensor_copy(slot[0:64, 512:1024], slot[0:64, 0:512])
            nc.vector.tensor_copy(slot[0:64, 1024:2048], slot[0:64, 0:1024])
            nc.vector.tensor_copy(slot[0:64, 2048:4096], slot[0:64, 0:2048])

        def evict(psum, cols_out, dest, qpair, bias, agg, off=0):
            """psum cols [off, off+512) -> dest[:, cols_out:+512] bf16 with
            relu+bias. Per-pair bias; even half ACT, odd half DVE.
            agg: optional accum target (cols qpair, qpair+1)."""
            acc0 = agg[:, qpair : qpair + 1] if agg is not None else None
            acc1 = agg[:, qpair + 1 : qpair + 2] if agg is not None else None
            nc.scalar.activation(
                dest[:, cols_out : cols_out + 256],
                psum[:, off : off + 256],
                AF.Relu,
                bias=bias[:, qpair : qpair + 1],
                accum_out=acc0,
            )
            nc.vector.scalar_tensor_tensor(
                dest[:, cols_out + 256 : cols_out + 512],
                psum[:, off + 256 : off + 512],
                bias[:, qpair + 1 : qpair + 2],
                zeros[:],
                op0=ALU.add,
                op1=ALU.max,
                accum_out=acc1,
            )

        # ================= PASS A: layer 0 =================
        seed_xpart(m2r[0], x0t2)
        seed_xpart(m2r[1], x0t2)
        for c in range(NCHUNK):
            if c == 0:
                fm = fm0
            else:
                fm = fmp.tile([128, QC * 256], _DT.bfloat16, tag="fm")
                nc.gpsimd.dma_start(
                    fm[:], e0d[:, c * QC * 256 : (c + 1) * QC * 256]
                )
            slot = m2r[c % 2]
            if c > 0:
                nc.sync.dma_start(
                    slot[64:66, :],
                    am1d[:, c * QC * 256 : (c + 1) * QC * 256],
                )

            for kk in range(QC // 4):  # 1024-col block-pairs in this chunk
                k = c * (QC // 4) + kk
                ps = pspB.tile([128, 1024], _DT.float32, tag="psB",
                               name=f"psA_{k}")
                for j in range(2):
                    gg = kk * 2 + j
                    nc.tensor.matmul(
                        ps[:, j * 512 : (j + 1) * 512], bd[0][:],
                        fm[:, gg * 512 : (gg + 1) * 512],
                        start=True, stop=False,
                    )
                    nc.tensor.matmul(
                        ps[:, j * 512 : (j + 1) * 512], w23_0[:],
                        slot[0:66, gg * 512 : (gg + 1) * 512],
                        start=False, stop=True,
                    )
                for j in range(2):
                    g = k * 2 + j
                    evict(ps[:, j * 512 : (j + 1) * 512].keep_view()
                          if False else ps,
                          g * 512, e1, 2 * g, bias0, aggP, off=j * 512)

        # ================= x1 / per-layer aux =================
        aggs = smallp.tile([128, 128], _DT.bfloat16, tag="aggs")
        nc.vector.tensor_mul(aggs[:], aggP[:], dinvP[:])

        psxa = pspB.tile([64, 128], _DT.float32, tag="psB")
        nc.tensor.matmul(
            psxa[:], wn0x[:], x0t2[:, 0:128], start=True, stop=False
        )
        nc.tensor.matmul(
            psxa[:], wn0a[:], aggs[0:64, :], start=False, stop=True
        )
        psxb = pspB.tile([64, 128], _DT.float32, tag="psB")
        nc.tensor.matmul(
            psxb[:], wn0x[:], x0t2[:, 128:256], start=True, stop=False
        )
        nc.tensor.matmul(
            psxb[:], wn0a2[64:128, :], aggs[64:128, :],
            start=False, stop=True,
        )
        nc.scalar.activation(
            x1t2[:, 0:128], psxa[:], AF.Relu, bias=bn0c[:, 0:1]
        )
        nc.scalar.activation(
            x1t2[:, 128:256], psxb[:], AF.Relu, bias=bn0c[:, 0:1]
        )
        nc.vector.tensor_copy(x1t2[:, 256:512], x1t2[:, 0:256])
        nc.vector.tensor_copy(x1o[0:64, :], x1t2[:, 0:256])

        # blt[p = r*64+g, f + 64*half] = Axi[f, 2g+r + 128*half] + be:
        # built from (r, g)-major column-gathered x1 (materialized once)
        x1g = smallp.tile([65, 256], _DT.bfloat16, tag="x1g")
        for h in range(2):
            nc.vector.tensor_copy(
                x1g[:, 128 * h : 128 * h + 128].rearrange(
                    "k (r g) -> k r g", r=2
                ),
                x1o[:, 128 * h : 128 * h + 128].rearrange(
                    "k (g r) -> k r g", r=2
                ),
            )
        x1oa = x1g[:, 0:128]
        x1ob = x1g[:, 128:256]
        for li, wxibe, w23r in ((0, wxibe1, w23r1), (1, wxibe2, w23r2)):
            psbl_a = pspB.tile([128, 64], _DT.float32, tag="psB",
                               name=f"psbla{li}")
            nc.tensor.matmul(psbl_a[:], x1oa, wxibe[:], start=True, stop=True)
            psbl_b = pspB.tile([128, 64], _DT.float32, tag="psB",
                               name=f"psblb{li}")
            nc.tensor.matmul(psbl_b[:], x1ob, wxibe[:], start=True, stop=True)
            blt = smallp.tile([128, 128], _DT.bfloat16, tag=f"blt{li}",
                              name=f"blt{li}")
            nc.scalar.activation(blt[:, 0:64], psbl_a[:], AF.Copy)
            nc.scalar.activation(blt[:, 64:128], psbl_b[:], AF.Copy)
            for r in range(2):
                nc.sync.dma_start(
                    w23r[66 + r : 67 + r, :],
                    blt[64 * r : 64 * r + 64, :],
                )

        # ================= PASS B: layers 1+2, skewed pipeline =================
        seed_xpart(m2r[0], x1t2)
        e2tiles = {}
        slots_b = {}

        def evict1024(psum, dest, k, acc, parity=0):
            """[128,1024] bias-free relu eviction; alternate engines."""
            accap = acc[:, k : k + 1] if acc is not None else None
            if (k + parity) % 2 == 0:
                nc.scalar.activation(
                    dest[:], psum[:], AF.Relu, accum_out=accap
                )
            else:
                nc.vector.tensor_scalar(
                    dest[:], psum[:], 0.0, 0.0,
                    op0=ALU.max, op1=ALU.add, accum_out=accap,
                )

        def mmpair(ps, lhs_e, rhs_e, w23r, slot, k):
            """Two [*,512] matmul groups into one [128,1024] psum tile."""
            for j in range(2):
                g = 2 * k + j
                nc.tensor.matmul(
                    ps[:, j * 512 : (j + 1) * 512], lhs_e,
                    rhs_e[:, j * 512 : (j + 1) * 512],
                    start=True, stop=False,
                )
                nc.tensor.matmul(
                    ps[:, j * 512 : (j + 1) * 512],
                    w23r[:, g * 128 : (g + 1) * 128],
                    slot[:, (g % 8) * 512 : (g % 8 + 1) * 512],
                    start=False, stop=True,
                )

        def stage_l1(k):
            g0 = 2 * k
            if g0 % 8 == 0:
                slot = m2r[(g0 // 8) % 2]
                nc.sync.dma_start(
                    slot[64:66, :],
                    am1d[:, g0 * 512 : g0 * 512 + QC * 256],
                )
                slots_b[g0 // 8] = slot
            slot = slots_b[g0 // 8]
            ps1 = pspB.tile([128, 1024], _DT.float32, tag="psB", name=f"psB1_{k}")
            mmpair(ps1, bd[1][:], e1[:, g0 * 512 : (g0 + 2) * 512],
                   w23r1, slot, k)
            e2s = e2p.tile([128, 1024], _DT.bfloat16, tag="e2s",
                           name=f"e2s_{k}")
            evict1024(ps1, e2s, k, None)
            e2tiles[k] = e2s

        def stage_l2(k):
            g0 = 2 * k
            slot = slots_b[g0 // 8]
            e2s = e2tiles.pop(k)
            ps2 = pspB.tile([128, 1024], _DT.float32, tag="psB", name=f"psB2_{k}")
            mmpair(ps2, bd[2][:], e2s[:], w23r2, slot, k)
            e3s = e3p.tile([128, 1024], _DT.bfloat16, tag="e3s",
                           name=f"e3s_{k}")
            evict1024(ps2, e3s, k, vcols, parity=1)

        SKEW = 1
        for k in range(NBLK // 2 + SKEW):
            if k == 1:
                seed_xpart(m2r[1], x1t2)
            if k < NBLK // 2:
                stage_l1(k)
            if k >= SKEW:
                stage_l2(k - SKEW)

        vcp = smallp.tile([128, 32], _DT.float32, tag="vcp")
        nc.vector.tensor_copy(vcp[:], vcols[:])
        nc.sync.dma_start(voutd, vcp[:])

    nc.compile()
    return nc


def _get_nc():
    global _nc_cache
    if _nc_cache is None:
        _nc_cache = _build_program()
    return _nc_cache


def _prep_core_inputs(b, edge_index, x, edge_attr, weights):
    (We0, be0, Wn0, bn0, We1, be1, We2, be2) = weights
    A = edge_index[b].astype(F32)
    x0 = x[b].astype(F32)

    A2 = A.reshape(2, 128, 256)                       # [r, q, j]
    am1 = (A2 - 1.0).reshape(2, NPAIR * 256).astype(BF16)

    x0t = x0.T.astype(F32)                            # [64, 256]
    x0t2 = np.tile(x0t, (1, 2)).astype(BF16)

    Axi0 = (x0 @ We0[0:64]).T + be0[:, None]          # [64, 256]
    bias0 = np.concatenate([Axi0[:, 0:128], Axi0[:, 128:256]], 0).astype(F32)

    deg = np.clip(A.sum(1), 1.0, None)
    dinv = (1.0 / deg).astype(F32)
    dinvP = np.concatenate(
        [np.tile(dinv[None, 0:128], (64, 1)), np.tile(dinv[None, 128:256], (64, 1))], 0
    ).astype(F32)

    def bdiag(We):
        Wee = We[128:192]
        out = np.zeros((128, 128), F32)
        out[0:64, 0:64] = Wee
        out[64:128, 64:128] = Wee
        return out.astype(BF16)

    big2 = np.zeros((2, 128), F32)
    big2[0, 0:64] = BIGV
    big2[1, 64:128] = BIGV

    def w23(We, masked):
        wxj2 = np.tile(We[64:128], (1, 2))
        rows = big2 if masked else np.zeros((2, 128), F32)
        return np.concatenate([wxj2, rows], 0).astype(BF16)

    def w23rep(We, masked):
        base = w23(We, masked).astype(F32)          # [66, 128]
        rep = np.tile(base, (1, 64))                # [66, 8192]
        out = np.zeros((68, 8192), F32)
        out[0:66] = rep
        return out.astype(BF16)

    ind2 = np.zeros((2, QC * 256), F32)
    ind2[0].reshape(8, 512)[:, 0:256] = 1.0
    ind2[1].reshape(8, 512)[:, 256:512] = 1.0

    return {
        # host pre-arrangement into feature-major pair-tiles:
        # e0[r*64+f, q*256+j] = edge_attr[q+128r, j, f]
        "e0": np.ascontiguousarray(
            edge_attr[b].astype(F32)
            .reshape(2, 128, 256, FE)
            .transpose(0, 3, 1, 2)
            .reshape(128, 128 * 256)
        ),
        "am1": am1,
        "x0t2": x0t2,
        "bias0": bias0,
        "dinvP": dinvP,
        "bd0": bdiag(We0),
        "bd1": bdiag(We1),
        "bd2": bdiag(We2),
        "w23_0": w23(We0, True),
        "w23rep_1": w23rep(We1, False),
        "w23rep_2": w23rep(We2, True),
        "ind2": ind2.astype(BF16),
        "wxibe1": np.concatenate([We1[0:64], be1[None, :]], 0).astype(BF16),
        "wxibe2": np.concatenate([We2[0:64], be2[None, :]], 0).astype(BF16),
        "wn0x": Wn0[0:64].astype(BF16),
        "wn0a": Wn0[64:128].astype(BF16),
        "wn0a2": np.concatenate([np.zeros((64, 64), F32), Wn0[64:128]], 0).astype(BF16),
        "bn0c": bn0[:, None].astype(F32),
    }


def run_traced(edge_index, x, edge_attr,
               We0, be0, Wn0, bn0,
               We1, be1, Wn1, bn1,
               We2, be2, Wn2, bn2,
               W1, b1, W2, b2, W3, b3, **kw):
    """Correctness + profiling run; returns (out, BassKernelResults)."""
    nc = _get_nc()
    weights = tuple(
        np.asarray(w, F32)
        for w in (We0, be0, Wn0, bn0, We1, be1, We2, be2)
    )
    in_maps = [
        _prep_core_inputs(b, np.asarray(edge_index), np.asarray(x),
                          np.asarray(edge_attr), weights)
        for b in range(B)
    ]
    res = run_bass_kernel_spmd(
        nc, in_maps, core_ids=list(range(B)), trace=True
    )
    return res


def kernel(edge_index, x, edge_attr,
           We0, be0, Wn0, bn0,
           We1, be1, Wn1, bn1,
           We2, be2, Wn2, bn2,
           W1, b1, W2, b2, W3, b3, **kw):
    nc = _get_nc()
    weights = tuple(
        np.asarray(w, F32)
        for w in (We0, be0, Wn0, bn0, We1, be1, We2, be2)
    )
    in_maps = [
        _prep_core_inputs(b, np.asarray(edge_index), np.asarray(x),
                          np.asarray(edge_attr), weights)
        for b in range(B)
    ]
    res = run_bass_kernel_spmd(nc, in_maps, core_ids=list(range(B)))
    out = np.zeros((B,), F32)
    for b in range(B):
        vc = res.results[b]["vcols"].astype(F32)
        v128 = vc.sum(1)
        v = (v128[:64] + v128[64:]) / float(N * N)
        h = _relu(v @ np.asarray(W1, F32) + np.asarray(b1, F32))
        h = _relu(h @ np.asarray(W2, F32) + np.asarray(b2, F32))
        out[b] = (h @ np.asarray(W3, F32) + np.asarray(b3, F32))[0]
    return out



# revision 5
# speedup vs baseline: 1.0277x; 1.0277x over previous
"""Trainium2 Bass kernel for nn_Disc_edge_15573551415682 (GNN message passing).

Sharding: data-parallel over batch B=8 -> 8 NeuronCores (1 graph/core).

Per graph, edge tensors live in "pair-tile" layout:
  pair q in [0,128) covers node rows (q, q+128); tile rows p = f + 64r hold
  feature f of row q+128r; tile cols are the neighbor index j.

Per layer, per pair q, ONE fp8 DoubleRow matmul computes the whole edge
update into PSUM [128, 256]:
  MM_ex  (K=128, 2 k-tiles): ktile0 = e-pair window of the e-arena
         (weights blockdiag(We_e; We_e)), ktile1 = shared xblock
         (x^T on rows 0:64, weights [Wxj|Wxj]; rows 64:128 zero).
         The two k-tiles address the SAME arena tile via a per-pair
         stride so ktile1 always lands on the shared xblock columns.
  MM_aux (K=2, 2 k-tiles): rhs strip rows = (A-1 mask row, ones row);
         weights = (BIG=240 mask pattern, per-pair bias Axi+be).
         Masked cols get -240 before relu -> exact 0; the ones row adds
         the sender-node bias, so evictions are bias-free.

Evictions are [128, 2048] relu-only chunks (8 pairs) split across
ACT/DVE, writing fp8 e-arenas. Layer-0 row-sums (agg) come from a bf16
fold-tree over the fp8 e1-arena on DVE (2x all-SBUF rate). Layer-2
eviction accum_out yields the masked column sums; the tiny head MLP
runs on host.
"""

import sys
from contextlib import ExitStack

import numpy as np

sys.path.insert(0, "/opt/trn_rl_repo")

import ml_dtypes  # noqa: E402

import concourse.bacc as bacc  # noqa: E402
import concourse.bass as bass  # noqa: E402
import concourse.tile as tile  # noqa: E402
from concourse import mybir  # noqa: E402
from concourse.bass_utils import run_bass_kernel_spmd  # noqa: E402

BF16 = ml_dtypes.bfloat16
F8 = ml_dtypes.float8_e4m3
F32 = np.float32

B, N, FN, FE = 8, 256, 64, 64
NPAIR = 128
CH = 8                 # pairs per chunk
NCHUNK = NPAIR // CH   # 16
AW = NPAIR * 256       # 32768 e-arena cols
ANC = AW + 256         # + shared xblock slot
BIG = 240.0

_DT = mybir.dt
AP = bass.AP
_nc_cache = None

# eviction engine maps per layer ('A' = ACT, 'V' = DVE), tuned vs sim
L0E = ["A"] * NCHUNK
for _i in (4, 9):
    L0E[_i] = "V"
L1E = ["A"] * NCHUNK
L2E = ["V"] * NCHUNK
for _i in (7, 15):
    L2E[_i] = "A"


def _relu(a):
    return np.maximum(a, 0.0)


def _build_program():
    nc = bacc.Bacc(
        "TRN2", target_bir_lowering=False, debug=False, num_devices=8
    )

    def din(name, shape, dt):
        return nc.dram_tensor(name, shape, dt, kind="ExternalInput").ap()

    e0d = din("e0", [128, ANC], _DT.float8e4)
    wexd = [din(f"wex{l}", [128, 256], _DT.float8e4) for l in range(3)]
    auxrd = din("auxr", [6, 22016], _DT.float8e4)
    auxwd = [din(f"auxw{l}", [6, 11008], _DT.float8e4) for l in range(3)]
    cbd = din("cb", [128, 768], _DT.bfloat16)
    vaccd = nc.dram_tensor(
        "vacc", [128, NCHUNK], _DT.float32, kind="ExternalOutput"
    ).ap()

    AF = mybir.ActivationFunctionType
    ALU = mybir.AluOpType
    PM = mybir.MatmulPerfMode.DoubleRow

    with tile.TileContext(nc) as tc, ExitStack() as ctx:
        cst = ctx.enter_context(tc.tile_pool(name="cst", bufs=1))
        arp = ctx.enter_context(tc.tile_pool(name="ar", bufs=1))
        psp = ctx.enter_context(tc.tile_pool(name="psB", bufs=2, space="PSUM"))
        fsc = ctx.enter_context(tc.tile_pool(name="fsc", bufs=2))
        deadp = ctx.enter_context(tc.tile_pool(name="dead", bufs=2))
        smallp = ctx.enter_context(tc.tile_pool(name="small", bufs=1))

        # ---------------- input tiles ----------------
        e0a = arp.tile([128, ANC], _DT.float8e4, tag="e0a", name="e0a")
        e1a = arp.tile([128, ANC], _DT.float8e4, tag="e1a", name="e1a")
        e2a = arp.tile([128, ANC], _DT.float8e4, tag="e2a", name="e2a")

        # e0 chunks via SWDGE (gpsimd), xblock0 tail + consts via HWDGE
        for c in range(NCHUNK // 2):
            nc.gpsimd.dma_start(
                e0a[:, c * 4096 : (c + 1) * 4096],
                e0d[:, c * 4096 : (c + 1) * 4096],
            )
        nc.sync.dma_start(e0a[:, AW:ANC], e0d[:, AW:ANC])

        wex = []
        for l in range(3):
            t = cst.tile([128, 2, 128], _DT.float8e4, tag=f"wex{l}",
                         name=f"wex{l}")
            nc.sync.dma_start(t[:].rearrange("p a b -> p (a b)"), wexd[l])
            wex.append(t)

        auxr = cst.tile([128, 22016], _DT.float8e4, tag="auxr", name="auxr")
        auxw = [
            cst.tile([128, 11008], _DT.float8e4, tag=f"auxw{l}",
                     name=f"auxw_{l}")
            for l in range(3)
        ]
        for g in range(3):
            nc.sync.dma_start(auxr[32 * g : 32 * g + 2, :],
                              auxrd[2 * g : 2 * g + 2, :])
            for l in range(3):
                nc.sync.dma_start(auxw[l][32 * g : 32 * g + 2, :],
                                  auxwd[l][2 * g : 2 * g + 2, :])

        cb = cst.tile([128, 768], _DT.bfloat16, tag="cb", name="cb")
        nc.sync.dma_start(cb[:], cbd)
        x0t = cb[0:64, 0:256]
        wn0x = cb[0:64, 256:320]
        wn0a = cb[0:64, 320:384]
        wn0a2 = cb[:, 384:448]
        dinvp = cb[:, 448:576]
        wxibe = [cb[0:65, 576:640], cb[0:65, 640:704]]
        bn0c = cb[0:64, 704:705]

        # ---------------- scratch / state ----------------
        foldbuf = smallp.tile([128, 2048], _DT.bfloat16, tag="foldbuf")
        aggv = smallp.tile([128, 128], _DT.bfloat16, tag="aggv")
        aggs = smallp.tile([128, 128], _DT.bfloat16, tag="aggs")
        x1o = smallp.tile([65, 256], _DT.bfloat16, tag="x1o")
        vacc = smallp.tile([128, NCHUNK], _DT.float32, tag="vacc")
        axt = smallp.tile([128, 256], _DT.float8e4, tag="axt")  # 4x [128,64]

        nc.vector.memset(x1o[64:65, :], 1.0)
        nc.vector.memset(e1a[64:128, AW:ANC], 0.0)

        def mm_pair(ps_slice, arena, wexl, auxwl, q):
            g = min(q // 43, 2)
            s = q - 43 * g
            rhs = AP(arena[:].tensor, AW,
                     [[ANC, 128], [q * 256 - AW, 2], [1, 256]])
            nc.tensor.matmul(ps_slice, wexl[:], rhs,
                             start=True, stop=False, perf_mode=PM)
            lw = AP(auxwl[:].tensor, 32 * g * 11008 + s * 256,
                    [[11008, 2], [128, 2], [1, 128]])
            lr = AP(auxr[:].tensor, 32 * g * 22016 + s * 512,
                    [[22016, 2], [256, 2], [1, 256]])
            nc.tensor.matmul(ps_slice, lw, lr,
                             start=False, stop=True, perf_mode=PM)

        def evict(eng, dst, ps, acc=None):
            if eng == "A":
                nc.scalar.activation(dst, ps[:], AF.Relu, accum_out=acc)
            else:
                nc.vector.tensor_scalar(dst, ps[:], 0.0, 0.0,
                                        op0=ALU.max, op1=ALU.add,
                                        accum_out=acc)

        def fold(dst, src0, src1):
            nc.vector.scalar_tensor_tensor(dst, src0, 0.0, src1,
                                           op0=ALU.add, op1=ALU.add)

        # ================= layer 0 =================
        for c in range(NCHUNK):
            ps = psp.tile([128, 2048], _DT.float32, tag="psB", name=f"ps0_{c}")
            for s in range(CH):
                mm_pair(ps[:, s * 256 : (s + 1) * 256], e0a, wex[0],
                        auxw[0], c * CH + s)
            evict(L0E[c], e1a[:, c * 2048 : (c + 1) * 2048], ps)

            # per-chunk fold tree: [8 pairs, 256] -> foldbuf [8 pairs, 16]
            t1 = fsc.tile([128, 1024], _DT.bfloat16, tag="t1", name=f"t1_{c}")
            va = AP(e1a[:].tensor, c * 2048, [[ANC, 128], [256, 8], [1, 128]])
            vb = AP(e1a[:].tensor, c * 2048 + 128,
                    [[ANC, 128], [256, 8], [1, 128]])
            fold(t1[:], va, vb)
            t2 = fsc.tile([128, 512], _DT.bfloat16, tag="t2", name=f"t2_{c}")
            fold(t2[:],
                 AP(t1[:].tensor, 0, [[1024, 128], [128, 8], [1, 64]]),
                 AP(t1[:].tensor, 64, [[1024, 128], [128, 8], [1, 64]]))
            t3 = fsc.tile([128, 256], _DT.bfloat16, tag="t3", name=f"t3_{c}")
            fold(t3[:],
                 AP(t2[:].tensor, 0, [[512, 128], [64, 8], [1, 32]]),
                 AP(t2[:].tensor, 32, [[512, 128], [64, 8], [1, 32]]))
            fold(AP(foldbuf[:].tensor, c * 128, [[2048, 128], [16, 8], [1, 16]]),
                 AP(t3[:].tensor, 0, [[256, 128], [32, 8], [1, 16]]),
                 AP(t3[:].tensor, 16, [[256, 128], [32, 8], [1, 16]]))

        # tail folds: foldbuf [128 pairs, 16] -> aggv [128, 128]
        ft1 = fsc.tile([128, 1024], _DT.bfloat16, tag="t1", name="ft1")
        fold(ft1[:],
             AP(foldbuf[:].tensor, 0, [[2048, 128], [16, 128], [1, 8]]),
             AP(foldbuf[:].tensor, 8, [[2048, 128], [16, 128], [1, 8]]))
        ft2 = fsc.tile([128, 512], _DT.bfloat16, tag="t2", name="ft2")
        fold(ft2[:],
             AP(ft1[:].tensor, 0, [[1024, 128], [8, 128], [1, 4]]),
             AP(ft1[:].tensor, 4, [[1024, 128], [8, 128], [1, 4]]))
        ft3 = fsc.tile([128, 256], _DT.bfloat16, tag="t3", name="ft3")
        fold(ft3[:],
             AP(ft2[:].tensor, 0, [[512, 128], [4, 128], [1, 2]]),
             AP(ft2[:].tensor, 2, [[512, 128], [4, 128], [1, 2]]))
        fold(aggv[:],
             AP(ft3[:].tensor, 0, [[256, 128], [2, 128], [1, 1]]),
             AP(ft3[:].tensor, 1, [[256, 128], [2, 128], [1, 1]]))

        # ================= x1 + aux strips for layers 1/2 =================
        nc.vector.tensor_tensor(aggs[:], aggv[:], dinvp, op=ALU.mult)

        psxa = psp.tile([64, 128], _DT.float32, tag="psB", name="psxa")
        nc.tensor.matmul(psxa[:], wn0x, x0t[:, 0:128], start=True, stop=False)
        nc.tensor.matmul(psxa[:], wn0a, aggs[0:64, :], start=False, stop=True)
        psxb = psp.tile([64, 128], _DT.float32, tag="psB", name="psxb")
        nc.tensor.matmul(psxb[:], wn0x, x0t[:, 128:256], start=True, stop=False)
        nc.tensor.matmul(psxb[:], wn0a2[64:128, :], aggs[64:128, :],
                         start=False, stop=True)
        # x1^T -> e1a xblock slot (fp8) and x1o (bf16)
        nc.scalar.activation(e1a[0:64, AW : AW + 128], psxa[:], AF.Relu,
                             bias=bn0c)
        nc.scalar.activation(e1a[0:64, AW + 128 : AW + 256], psxb[:], AF.Relu,
                             bias=bn0c)
        nc.scalar.activation(x1o[0:64, 0:128], psxa[:], AF.Relu, bias=bn0c)
        nc.scalar.activation(x1o[0:64, 128:256], psxb[:], AF.Relu, bias=bn0c)
        nc.vector.tensor_copy(e2a[:, AW:ANC], e1a[:, AW:ANC])

        # per-layer bias strips: Axi_l + be_l, fp8, scattered into auxw[l]
        for li in (1, 2):
            pslo = psp.tile([128, 64], _DT.float32, tag="psB", name=f"pl{li}")
            nc.tensor.matmul(pslo[:], x1o[:, 0:128], wxibe[li - 1],
                             start=True, stop=True)
            pshi = psp.tile([128, 64], _DT.float32, tag="psB", name=f"ph{li}")
            nc.tensor.matmul(pshi[:], x1o[:, 128:256], wxibe[li - 1],
                             start=True, stop=True)
            lo = axt[:, (li - 1) * 128 : (li - 1) * 128 + 64]
            hi = axt[:, (li - 1) * 128 + 64 : li * 128]
            nc.scalar.activation(lo, pslo[:], AF.Copy)
            nc.scalar.activation(hi, pshi[:], AF.Copy)
            for h in (0, 1):
                for g, q0, gn in ((0, 0, 43), (1, 43, 43), (2, 86, 42)):
                    nc.sync.dma_start(
                        AP(auxw[li][:].tensor,
                           (32 * g + 1) * 11008 + h * 64,
                           [[11008, 1], [256, gn], [1, 64]]),
                        axt[q0 : q0 + gn,
                            (li - 1) * 128 + h * 64 :
                            (li - 1) * 128 + h * 64 + 64],
                    )

        # ================= layers 1 and 2 =================
        for c in range(NCHUNK):
            ps = psp.tile([128, 2048], _DT.float32, tag="psB", name=f"ps1_{c}")
            for s in range(CH):
                mm_pair(ps[:, s * 256 : (s + 1) * 256], e1a, wex[1],
                        auxw[1], c * CH + s)
            evict(L1E[c], e2a[:, c * 2048 : (c + 1) * 2048], ps)

        for c in range(NCHUNK):
            ps = psp.tile([128, 2048], _DT.float32, tag="psB", name=f"ps2_{c}")
            for s in range(CH):
                mm_pair(ps[:, s * 256 : (s + 1) * 256], e2a, wex[2],
                        auxw[2], c * CH + s)
            dead = deadp.tile([128, 2048], _DT.float8e4, tag="dead",
                              name=f"dead_{c}")
            evict(L2E[c], dead[:], ps, acc=vacc[:, c : c + 1])

        nc.sync.dma_start(vaccd, vacc[:])

    nc.compile()
    return nc


def _get_nc():
    global _nc_cache
    if _nc_cache is None:
        _nc_cache = _build_program()
    return _nc_cache


def _prep_core_inputs(b, edge_index, x, edge_attr, weights):
    (We0, be0, Wn0, bn0, We1, be1, We2, be2) = weights
    A = edge_index[b].astype(F32)
    x0 = x[b].astype(F32)

    # e0 arena + xblock0
    e0 = np.empty((128, ANC), F32)
    e0[:, 0:AW] = (
        edge_attr[b].astype(F32)
        .reshape(2, 128, 256, FE)
        .transpose(0, 3, 1, 2)
        .reshape(128, AW)
    )
    e0[:, AW:ANC] = 0.0
    e0[0:64, AW:ANC] = x0.T

    def mk_wex(We):
        # ktile0 = shared xblock (Wxj), ktile1 = e-pair window (We_e)
        out = np.zeros((128, 2, 128), F32)
        Wee = We[128:192]
        out[0:64, 0, 0:64] = We[64:128]
        out[0:64, 0, 64:128] = We[64:128]
        out[0:64, 1, 0:64] = Wee
        out[64:128, 1, 64:128] = Wee
        return out.reshape(128, 256).astype(F8)

    # aux rhs strip: row 2g = (A-1) interleaved, row 2g+1 = ones
    GQ0 = (0, 43, 86)
    GN = (43, 43, 42)
    auxr = np.zeros((6, 22016), F32)
    for g in range(3):
        qs = np.arange(GQ0[g], GQ0[g] + GN[g])
        blk = np.stack([A[qs] - 1.0, A[qs + 128] - 1.0], axis=1)
        auxr[2 * g, 0 : GN[g] * 512] = blk.reshape(-1)
        auxr[2 * g + 1] = 1.0

    def mk_auxw(masked, bias):
        # bias: [256, 64] (Axi + be) or None
        out = np.zeros((6, 43, 2, 128), F32)
        if masked:
            out[0::2, :, 0, 0:64] = BIG
            out[0::2, :, 1, 64:128] = BIG
        if bias is not None:
            for g in range(3):
                qs = np.arange(GQ0[g], GQ0[g] + GN[g])
                out[2 * g + 1, 0 : GN[g], 0, 0:64] = bias[qs]
                out[2 * g + 1, 0 : GN[g], 0, 64:128] = bias[qs + 128]
        return out.reshape(6, 11008).astype(F8)

    bias0 = x0 @ We0[0:64] + be0[None, :]

    deg = np.clip(A.sum(1), 1.0, None)
    dinv = (1.0 / deg).astype(F32)
    dinvp = np.concatenate(
        [np.tile(dinv[None, 0:128], (64, 1)),
         np.tile(dinv[None, 128:256], (64, 1))], 0
    )

    cb = np.zeros((128, 768), F32)
    cb[0:64, 0:256] = x0.T
    cb[0:64, 256:320] = Wn0[0:64]
    cb[0:64, 320:384] = Wn0[64:128]
    cb[64:128, 384:448] = Wn0[64:128]
    cb[:, 448:576] = dinvp
    cb[0:64, 576:640] = We1[0:64]
    cb[64, 576:640] = be1
    cb[0:64, 640:704] = We2[0:64]
    cb[64, 640:704] = be2
    cb[0:64, 704] = bn0

    return {
        "e0": e0.astype(F8),
        "wex0": mk_wex(We0),
        "wex1": mk_wex(We1),
        "wex2": mk_wex(We2),
        "auxr": auxr.astype(F8),
        "auxw0": mk_auxw(True, bias0),
        "auxw1": mk_auxw(False, None),
        "auxw2": mk_auxw(True, None),
        "cb": cb.astype(BF16),
    }


def _finish(res, W1, b1, W2, b2, W3, b3):
    out = np.zeros((B,), F32)
    for b in range(B):
        vacc = res.results[b]["vacc"].astype(F32)
        vcols = vacc.sum(1)
        v = (vcols[0:64] + vcols[64:128]) / float(N * N)
        h = _relu(v @ np.asarray(W1, F32) + np.asarray(b1, F32))
        h = _relu(h @ np.asarray(W2, F32) + np.asarray(b2, F32))
        out[b] = (h @ np.asarray(W3, F32) + np.asarray(b3, F32))[0]
    return out


def _run(edge_index, x, edge_attr, weights, trace=False):
    nc = _get_nc()
    in_maps = [
        _prep_core_inputs(b, np.asarray(edge_index), np.asarray(x),
                          np.asarray(edge_attr), weights)
        for b in range(B)
    ]
    return run_bass_kernel_spmd(nc, in_maps, core_ids=list(range(B)),
                                trace=trace)


def run_traced(edge_index, x, edge_attr,
               We0, be0, Wn0, bn0,
               We1, be1, Wn1, bn1,
               We2, be2, Wn2, bn2,
               W1, b1, W2, b2, W3, b3, **kw):
    weights = tuple(
        np.asarray(w, F32)
        for w in (We0, be0, Wn0, bn0, We1, be1, We2, be2)
    )
    return _run(edge_index, x, edge_attr, weights, trace=True)


def kernel(edge_index, x, edge_attr,
           We0, be0, Wn0, bn0,
           We1, be1, Wn1, bn1,
           We2, be2, Wn2, bn2,
           W1, b1, W2, b2, W3, b3, **kw):
    weights = tuple(
        np.asarray(w, F32)
        for w in (We0, be0, Wn0, bn0, We1, be1, We2, be2)
    )
    res = _run(edge_index, x, edge_attr, weights)
    return _finish(res, W1, b1, W2, b2, W3, b3)


# revision 6
# speedup vs baseline: 1.1074x; 1.0775x over previous
"""Trainium2 Bass kernel for nn_Disc_edge_15573551415682 (GNN message passing).

Sharding: data-parallel over batch B=8 -> 8 NeuronCores (1 graph/core).

Per graph, edge tensors live in "pair-tile" layout:
  pair q in [0,128) covers node rows (q, q+128); tile rows p = f + 64r hold
  feature f of row q+128r; tile cols are the neighbor index j.

Per layer, per pair q, ONE fp8 DoubleRow matmul computes the whole edge
update into PSUM [128, 256]:
  MM_ex  (K=128, 2 k-tiles): ktile0 = e-pair window of the e-arena
         (weights blockdiag(We_e; We_e)), ktile1 = shared xblock
         (x^T on rows 0:64, weights [Wxj|Wxj]; rows 64:128 zero).
         The two k-tiles address the SAME arena tile via a per-pair
         stride so ktile1 always lands on the shared xblock columns.
  MM_aux (K=2, 2 k-tiles): rhs strip rows = (A-1 mask row, ones row);
         weights = (BIG=240 mask pattern, per-pair bias Axi+be).
         Masked cols get -240 before relu -> exact 0; the ones row adds
         the sender-node bias, so evictions are bias-free.

Evictions are [128, 2048] relu-only chunks (8 pairs) split across
ACT/DVE, writing fp8 e-arenas. Layer-0 row-sums (agg) come from a bf16
fold-tree over the fp8 e1-arena on DVE (2x all-SBUF rate). Layer-2
eviction accum_out yields the masked column sums; the tiny head MLP
runs on host.
"""

import sys
from contextlib import ExitStack

import numpy as np

sys.path.insert(0, "/opt/trn_rl_repo")

import ml_dtypes  # noqa: E402

import concourse.bacc as bacc  # noqa: E402
import concourse.bass as bass  # noqa: E402
import concourse.tile as tile  # noqa: E402
from concourse import mybir  # noqa: E402
from concourse.bass_utils import run_bass_kernel_spmd  # noqa: E402

BF16 = ml_dtypes.bfloat16
F8 = ml_dtypes.float8_e4m3
F32 = np.float32

B, N, FN, FE = 8, 256, 64, 64
NPAIR = 128
CH = 8                 # pairs per chunk
NCHUNK = NPAIR // CH   # 16
AW = NPAIR * 256       # 32768 e-arena cols
ANC = AW + 256         # + shared xblock slot
BIG = 240.0

_DT = mybir.dt
AP = bass.AP
_nc_cache = None

# eviction engine maps per layer ('A' = ACT, 'V' = DVE), tuned vs sim
L0E = ["A"] * NCHUNK
L1E = ["A"] * 14 + ["V"] * 2
L2E = ["A"] * 4 + ["V"] * 12


def _relu(a):
    return np.maximum(a, 0.0)


def _build_program():
    nc = bacc.Bacc(
        "TRN2", target_bir_lowering=False, debug=False, num_devices=8
    )

    def din(name, shape, dt):
        return nc.dram_tensor(name, shape, dt, kind="ExternalInput").ap()

    e0d = din("e0", [128, ANC], _DT.float8e4)
    wexd = [din(f"wex{l}", [128, 256], _DT.float8e4) for l in range(3)]
    auxrd = din("auxr", [6, 22016], _DT.float8e4)
    auxwd = [din(f"auxw{l}", [6, 11008], _DT.float8e4) for l in range(3)]
    cbd = din("cb", [128, 768], _DT.bfloat16)
    vaccd = nc.dram_tensor(
        "vacc", [128, NCHUNK], _DT.float32, kind="ExternalOutput"
    ).ap()

    AF = mybir.ActivationFunctionType
    ALU = mybir.AluOpType
    PM = mybir.MatmulPerfMode.DoubleRow

    with tile.TileContext(nc) as tc, ExitStack() as ctx:
        cst = ctx.enter_context(tc.tile_pool(name="cst", bufs=1))
        arp = ctx.enter_context(tc.tile_pool(name="ar", bufs=1))
        psp = ctx.enter_context(tc.tile_pool(name="psB", bufs=2, space="PSUM"))
        fsc = ctx.enter_context(tc.tile_pool(name="fsc", bufs=2))
        deadp = ctx.enter_context(tc.tile_pool(name="dead", bufs=2))
        smallp = ctx.enter_context(tc.tile_pool(name="small", bufs=1))

        # ---------------- input tiles ----------------
        e0a = arp.tile([128, ANC], _DT.float8e4, tag="e0a", name="e0a")
        e1a = arp.tile([128, ANC], _DT.float8e4, tag="e1a", name="e1a")
        e2a = arp.tile([128, ANC], _DT.float8e4, tag="e2a", name="e2a")

        # e0 chunks via SWDGE (gpsimd), xblock0 tail + consts via HWDGE
        for c in range(NCHUNK // 2):
            nc.gpsimd.dma_start(
                e0a[:, c * 4096 : (c + 1) * 4096],
                e0d[:, c * 4096 : (c + 1) * 4096],
            )
        nc.sync.dma_start(e0a[:, AW:ANC], e0d[:, AW:ANC])

        wex = []
        for l in range(3):
            t = cst.tile([128, 2, 128], _DT.float8e4, tag=f"wex{l}",
                         name=f"wex{l}")
            nc.sync.dma_start(t[:].rearrange("p a b -> p (a b)"), wexd[l])
            wex.append(t)

        auxr = cst.tile([128, 22016], _DT.float8e4, tag="auxr", name="auxr")
        auxw = [
            cst.tile([128, 11008], _DT.float8e4, tag=f"auxw{l}",
                     name=f"auxw_{l}")
            for l in range(3)
        ]
        for g in range(3):
            nc.sync.dma_start(auxr[32 * g : 32 * g + 2, :],
                              auxrd[2 * g : 2 * g + 2, :])
            for l in range(3):
                nc.sync.dma_start(auxw[l][32 * g : 32 * g + 2, :],
                                  auxwd[l][2 * g : 2 * g + 2, :])

        cb = cst.tile([128, 768], _DT.bfloat16, tag="cb", name="cb")
        nc.sync.dma_start(cb[:], cbd)
        x0t = cb[0:64, 0:256]
        wn0x = cb[0:64, 256:320]
        wn0a = cb[0:64, 320:384]
        wn0a2 = cb[:, 384:448]
        dinvp = cb[:, 448:576]
        wxibe = [cb[0:65, 576:640], cb[0:65, 640:704]]
        bn0c = cb[0:64, 704:705]

        # ---------------- scratch / state ----------------
        aggv = smallp.tile([128, 128], _DT.float32, tag="aggv")
        adum = smallp.tile([128, 256], _DT.bfloat16, tag="adum")
        aggs = smallp.tile([128, 128], _DT.bfloat16, tag="aggs")
        x1o = smallp.tile([65, 256], _DT.bfloat16, tag="x1o")
        vacc = smallp.tile([128, NCHUNK], _DT.float32, tag="vacc")
        axt = smallp.tile([128, 256], _DT.float8e4, tag="axt")  # 4x [128,64]

        nc.vector.memset(x1o[64:65, :], 1.0)
        nc.vector.memset(e1a[64:128, AW:ANC], 0.0)

        def mm_pair(ps_slice, arena, wexl, auxwl, q):
            g = min(q // 43, 2)
            s = q - 43 * g
            rhs = AP(arena[:].tensor, AW,
                     [[ANC, 128], [q * 256 - AW, 2], [1, 256]])
            nc.tensor.matmul(ps_slice, wexl[:], rhs,
                             start=True, stop=False, perf_mode=PM)
            lw = AP(auxwl[:].tensor, 32 * g * 11008 + s * 256,
                    [[11008, 2], [128, 2], [1, 128]])
            lr = AP(auxr[:].tensor, 32 * g * 22016 + s * 512,
                    [[22016, 2], [256, 2], [1, 256]])
            nc.tensor.matmul(ps_slice, lw, lr,
                             start=False, stop=True, perf_mode=PM)

        def evict(eng, dst, ps, acc=None):
            if eng == "A":
                nc.scalar.activation(dst, ps[:], AF.Relu, accum_out=acc)
            else:
                nc.vector.tensor_scalar(dst, ps[:], 0.0, 0.0,
                                        op0=ALU.max, op1=ALU.add,
                                        accum_out=acc)

        # ================= layer 0 =================
        for c in range(NCHUNK):
            ps = psp.tile([128, 2048], _DT.float32, tag="psB", name=f"ps0_{c}")
            for s in range(CH):
                mm_pair(ps[:, s * 256 : (s + 1) * 256], e0a, wex[0],
                        auxw[0], c * CH + s)
            evict(L0E[c], e1a[:, c * 2048 : (c + 1) * 2048], ps)

            # per-pair row-sum pass over the fp8 arena (DVE 2x all-SBUF)
            for s in range(CH):
                q = c * CH + s
                nc.vector.tensor_scalar(
                    adum[:], e1a[:, q * 256 : (q + 1) * 256], 0.0, 0.0,
                    op0=ALU.add, op1=ALU.add,
                    accum_out=aggv[:, q : q + 1])

        # ================= x1 + aux strips for layers 1/2 =================
        nc.vector.tensor_tensor(aggs[:], aggv[:], dinvp, op=ALU.mult)

        psxa = psp.tile([64, 128], _DT.float32, tag="psB", name="psxa")
        nc.tensor.matmul(psxa[:], wn0x, x0t[:, 0:128], start=True, stop=False)
        nc.tensor.matmul(psxa[:], wn0a, aggs[0:64, :], start=False, stop=True)
        psxb = psp.tile([64, 128], _DT.float32, tag="psB", name="psxb")
        nc.tensor.matmul(psxb[:], wn0x, x0t[:, 128:256], start=True, stop=False)
        nc.tensor.matmul(psxb[:], wn0a2[64:128, :], aggs[64:128, :],
                         start=False, stop=True)
        # x1^T -> e1a xblock slot (fp8) and x1o (bf16)
        nc.scalar.activation(e1a[0:64, AW : AW + 128], psxa[:], AF.Relu,
                             bias=bn0c)
        nc.scalar.activation(e1a[0:64, AW + 128 : AW + 256], psxb[:], AF.Relu,
                             bias=bn0c)
        nc.scalar.activation(x1o[0:64, 0:128], psxa[:], AF.Relu, bias=bn0c)
        nc.scalar.activation(x1o[0:64, 128:256], psxb[:], AF.Relu, bias=bn0c)
        nc.vector.tensor_copy(e2a[:, AW:ANC], e1a[:, AW:ANC])

        # per-layer bias strips: Axi_l + be_l, fp8, scattered into auxw[l]
        for li in (1, 2):
            pslo = psp.tile([128, 64], _DT.float32, tag="psB", name=f"pl{li}")
            nc.tensor.matmul(pslo[:], x1o[:, 0:128], wxibe[li - 1],
                             start=True, stop=True)
            pshi = psp.tile([128, 64], _DT.float32, tag="psB", name=f"ph{li}")
            nc.tensor.matmul(pshi[:], x1o[:, 128:256], wxibe[li - 1],
                             start=True, stop=True)
            lo = axt[:, (li - 1) * 128 : (li - 1) * 128 + 64]
            hi = axt[:, (li - 1) * 128 + 64 : li * 128]
            nc.scalar.activation(lo, pslo[:], AF.Copy)
            nc.scalar.activation(hi, pshi[:], AF.Copy)
            for h in (0, 1):
                for g, q0, gn in ((0, 0, 43), (1, 43, 43), (2, 86, 42)):
                    nc.sync.dma_start(
                        AP(auxw[li][:].tensor,
                           (32 * g + 1) * 11008 + h * 64,
                           [[11008, 1], [256, gn], [1, 64]]),
                        axt[q0 : q0 + gn,
                            (li - 1) * 128 + h * 64 :
                            (li - 1) * 128 + h * 64 + 64],
                    )

        # ================= layers 1 and 2 (skewed interleave) =================
        def l1_chunk(c):
            ps = psp.tile([128, 2048], _DT.float32, tag="psB", name=f"ps1_{c}")
            for s in range(CH):
                mm_pair(ps[:, s * 256 : (s + 1) * 256], e1a, wex[1],
                        auxw[1], c * CH + s)
            evict(L1E[c], e2a[:, c * 2048 : (c + 1) * 2048], ps)

        def l2_chunk(c):
            ps = psp.tile([128, 2048], _DT.float32, tag="psB", name=f"ps2_{c}")
            for s in range(CH):
                mm_pair(ps[:, s * 256 : (s + 1) * 256], e2a, wex[2],
                        auxw[2], c * CH + s)
            dead = deadp.tile([128, 2048], _DT.float8e4, tag="dead",
                              name=f"dead_{c}")
            evict(L2E[c], dead[:], ps, acc=vacc[:, c : c + 1])

        for c in range(NCHUNK + 1):
            if c < NCHUNK:
                l1_chunk(c)
            if c >= 1:
                l2_chunk(c - 1)

        nc.sync.dma_start(vaccd, vacc[:])

    nc.compile()
    return nc


def _get_nc():
    global _nc_cache
    if _nc_cache is None:
        _nc_cache = _build_program()
    return _nc_cache


def _prep_core_inputs(b, edge_index, x, edge_attr, weights):
    (We0, be0, Wn0, bn0, We1, be1, We2, be2) = weights
    A = edge_index[b].astype(F32)
    x0 = x[b].astype(F32)

    # e0 arena + xblock0
    e0 = np.empty((128, ANC), F32)
    e0[:, 0:AW] = (
        edge_attr[b].astype(F32)
        .reshape(2, 128, 256, FE)
        .transpose(0, 3, 1, 2)
        .reshape(128, AW)
    )
    e0[:, AW:ANC] = 0.0
    e0[0:64, AW:ANC] = x0.T

    def mk_wex(We):
        # ktile0 = shared xblock (Wxj), ktile1 = e-pair window (We_e)
        out = np.zeros((128, 2, 128), F32)
        Wee = We[128:192]
        out[0:64, 0, 0:64] = We[64:128]
        out[0:64, 0, 64:128] = We[64:128]
        out[0:64, 1, 0:64] = Wee
        out[64:128, 1, 64:128] = Wee
        return out.reshape(128, 256).astype(F8)

    # aux rhs strip: row 2g = (A-1) interleaved, row 2g+1 = ones
    GQ0 = (0, 43, 86)
    GN = (43, 43, 42)
    auxr = np.zeros((6, 22016), F32)
    for g in range(3):
        qs = np.arange(GQ0[g], GQ0[g] + GN[g])
        blk = np.stack([A[qs] - 1.0, A[qs + 128] - 1.0], axis=1)
        auxr[2 * g, 0 : GN[g] * 512] = blk.reshape(-1)
        auxr[2 * g + 1] = 1.0

    def mk_auxw(masked, bias):
        # bias: [256, 64] (Axi + be) or None
        out = np.zeros((6, 43, 2, 128), F32)
        if masked:
            out[0::2, :, 0, 0:64] = BIG
            out[0::2, :, 1, 64:128] = BIG
        if bias is not None:
            for g in range(3):
                qs = np.arange(GQ0[g], GQ0[g] + GN[g])
                out[2 * g + 1, 0 : GN[g], 0, 0:64] = bias[qs]
                out[2 * g + 1, 0 : GN[g], 0, 64:128] = bias[qs + 128]
        return out.reshape(6, 11008).astype(F8)

    bias0 = x0 @ We0[0:64] + be0[None, :]

    deg = np.clip(A.sum(1), 1.0, None)
    dinv = (1.0 / deg).astype(F32)
    dinvp = np.concatenate(
        [np.tile(dinv[None, 0:128], (64, 1)),
         np.tile(dinv[None, 128:256], (64, 1))], 0
    )

    cb = np.zeros((128, 768), F32)
    cb[0:64, 0:256] = x0.T
    cb[0:64, 256:320] = Wn0[0:64]
    cb[0:64, 320:384] = Wn0[64:128]
    cb[64:128, 384:448] = Wn0[64:128]
    cb[:, 448:576] = dinvp
    cb[0:64, 576:640] = We1[0:64]
    cb[64, 576:640] = be1
    cb[0:64, 640:704] = We2[0:64]
    cb[64, 640:704] = be2
    cb[0:64, 704] = bn0

    return {
        "e0": e0.astype(F8),
        "wex0": mk_wex(We0),
        "wex1": mk_wex(We1),
        "wex2": mk_wex(We2),
        "auxr": auxr.astype(F8),
        "auxw0": mk_auxw(True, bias0),
        "auxw1": mk_auxw(False, None),
        "auxw2": mk_auxw(True, None),
        "cb": cb.astype(BF16),
    }


def _finish(res, W1, b1, W2, b2, W3, b3):
    out = np.zeros((B,), F32)
    for b in range(B):
        vacc = res.results[b]["vacc"].astype(F32)
        vcols = vacc.sum(1)
        v = (vcols[0:64] + vcols[64:128]) / float(N * N)
        h = _relu(v @ np.asarray(W1, F32) + np.asarray(b1, F32))
        h = _relu(h @ np.asarray(W2, F32) + np.asarray(b2, F32))
        out[b] = (h @ np.asarray(W3, F32) + np.asarray(b3, F32))[0]
    return out


def _run(edge_index, x, edge_attr, weights, trace=False):
    nc = _get_nc()
    in_maps = [
        _prep_core_inputs(b, np.asarray(edge_index), np.asarray(x),
                          np.asarray(edge_attr), weights)
        for b in range(B)
    ]
    return run_bass_kernel_spmd(nc, in_maps, core_ids=list(range(B)),
                                trace=trace)


def run_traced(edge_index, x, edge_attr,
               We0, be0, Wn0, bn0,
               We1, be1, Wn1, bn1,
               We2, be2, Wn2, bn2,
               W1, b1, W2, b2, W3, b3, **kw):
    weights = tuple(
        np.asarray(w, F32)
        for w in (We0, be0, Wn0, bn0, We1, be1, We2, be2)
    )
    return _run(edge_index, x, edge_attr, weights, trace=True)


def kernel(edge_index, x, edge_attr,
           We0, be0, Wn0, bn0,
           We1, be1, Wn1, bn1,
           We2, be2, Wn2, bn2,
           W1, b1, W2, b2, W3, b3, **kw):
    weights = tuple(
        np.asarray(w, F32)
        for w in (We0, be0, Wn0, bn0, We1, be1, We2, be2)
    )
    res = _run(edge_index, x, edge_attr, weights)
    return _finish(res, W1, b1, W2, b2, W3, b3)
